# revision 19
# baseline (speedup 1.0000x reference)
"""Trainium2 Bass kernel for nn_Attention (ViT-style attention block).

Reference computation (per batch b, head h):
    qkv  = x @ qkv_weight.T + [q_bias, 0, v_bias]        # [B,N,3C]
    q,k,v split into heads of HD=64;  q *= HD**-0.5
    S    = q @ k.T + relative_position_bias[h]           # [N,N]
    P    = softmax(S, axis=-1)
    O    = P @ v                                         # [N,HD]
    out  = concat_heads(O) @ proj_weight.T + proj_bias   # [B,N,C]

Sharding: pure data-parallel over batch: 16 batches -> 2 per core across
8 NeuronCores; weights replicated; outputs concatenated on the host.

Single software-pipelined instruction stream (v2): qkv projection chains,
attention head-pair slots and the output projection are emitted in one
interleaved order so the PE never idles long enough to drop out of its
full-speed p-state (PE runs at 1.2GHz for ~3us after any idle gap; the
baseline's phase-separated version paid ~2x on every attention matmul).

Per attention slot (head-pair hp, batch b), queries split 512 + 65 so
every PSUM access stays inside one 2KB bank:
  - S^T pair matmuls (K=64) for both heads at tile_position (0,0)/(64,0).
  - ONE exp on ScalarE per mt covers both heads; P^T *= exp(bias^T)
    (host-precomputed) split between DVE and Pool.
  - O'^T accumulates over key tiles; v_aug = [1 | v] puts the softmax
    denominator in PSUM partition 0, so DVE reciprocal reads it directly
    (no SWDGE collect); f32 HWDGE bounce + partition-broadcast; normalization is an in-place multiply on Pool.
  - qkv/proj chains for FUTURE slots are emitted inside each slot as PE
    filler while ScalarE (exp, the per-slot rate limiter) catches up.

PSUM budget (8 banks): chain pool 2x[128,1024]f32 (4) + S-pair pool
2x[128,2,512]f32 (4); O accumulators borrow chain-pool tiles.

bf16 DMA rule learned on HW: a DMA write run into SBUF must start 4-byte
aligned and may overhang its end by 2 bytes -- all bf16 destination rows
here are padded so overhangs land in padding (580-wide bias rows, 578-wide
ot rows, 66-wide tail tiles).
"""

import numpy as np

B, N, C = 16, 577, 768
H, HD = 12, 64
SCALE = HD ** -0.5
NCORES = 8
BL = B // NCORES           # batches per core (2)
T = BL * N                 # tokens per core (1154)
NT_M = (N + 127) // 128    # key tiles per batch (5: 4x128 + 65)
CT = C // 128              # 128-contraction tiles over C (6)
HP = H // 2                # head pairs (6)
NP = N + 1                 # padded query stride (578)
TP = BL * NP               # padded token rows (1156)
NQ0 = 512                  # main-pass queries per batch
NQ1 = N - NQ0              # tail-pass queries (65)
NQ1W = 66                  # tail width incl pad query (4B-aligned blocks)
NBP = 580                  # bias row stride (577 padded to 4B-aligned even)

_CACHE = {}


def _chunks(total, limit=512):
    out = []
    pos = 0
    while pos < total:
        n = min(limit, total - pos)
        out.append((pos, n))
        pos += n
    return out


def _build():
    """Trace the Bass/Tile program once. Returns the Bass object."""
    import concourse.bass as bass
    import concourse.tile as tile
    from concourse import bacc, mybir
    from contextlib import ExitStack

    f32 = mybir.dt.float32
    bf16 = mybir.dt.bfloat16
    ALU = mybir.AluOpType
    ACTF = mybir.ActivationFunctionType

    nc = bacc.Bacc("TRN2", target_bir_lowering=False, debug=False)

    # ---- DRAM I/O ----
    xT_d = nc.dram_tensor("xT", [C, T], bf16, kind="ExternalInput").ap()
    # qk weights partition-major + jt-block-major: one contiguous DMA per jt
    wqk_d = nc.dram_tensor(
        "wqkH", [128, 2 * CT, CT, 128], bf16, kind="ExternalInput"
    ).ap()
    wv_d = nc.dram_tensor("wvT", [C, C], bf16, kind="ExternalInput").ap()
    wp_d = nc.dram_tensor("wpT", [C, C], bf16, kind="ExternalInput").ap()
    qb_d = nc.dram_tensor("qbT", [128, CT], f32, kind="ExternalInput").ap()
    vb_d = nc.dram_tensor("vbB", [128, C], f32, kind="ExternalInput").ap()
    pb_d = nc.dram_tensor("pbB", [128, C], f32, kind="ExternalInput").ap()
    bt_d = nc.dram_tensor("BT", [H, N, NBP], bf16, kind="ExternalInput").ap()
    out_d = nc.dram_tensor("out", [TP, C], bf16, kind="ExternalOutput").ap()

    with tile.TileContext(nc) as tc, ExitStack() as ctx:
        const = ctx.enter_context(tc.tile_pool(name="const", bufs=1))
        persist = ctx.enter_context(tc.tile_pool(name="persist", bufs=1))

        qb_sb = const.tile([128, CT], f32)
        vb_sb = const.tile([128, C], f32)
        pb_sb = const.tile([128, C], f32)

        xT_sb = persist.tile([128, CT, T], bf16)
        wqk_sb = persist.tile([128, 2 * CT, CT, 128], bf16)
        wv_sb = persist.tile([128, CT, C], bf16)
        wp_sb = persist.tile([128, CT, C], bf16)
        qk_sb = persist.tile([128, 2 * CT, BL, NP], bf16)   # q^T | k^T
        v_sb = persist.tile([128, BL * NT_M, H, HD + 2], bf16)  # [1|v|pad]
        ot_sb = persist.tile([128, HP, BL, NP], bf16)       # O^T normalized

        # SBUF pools
        btp = ctx.enter_context(tc.tile_pool(name="btp", bufs=2))
        ptp = ctx.enter_context(tc.tile_pool(name="ptp", bufs=12))
        scp = ctx.enter_context(tc.tile_pool(name="scp", bufs=3))
        rap = ctx.enter_context(tc.tile_pool(name="rap", bufs=2))
        rdp = ctx.enter_context(tc.tile_pool(name="rdp", bufs=2, space="DRAM"))
        bcp = ctx.enter_context(tc.tile_pool(name="bcp", bufs=3))
        outp = ctx.enter_context(tc.tile_pool(name="outp", bufs=3))

        # PSUM pools: chains+O accumulators (2-bank tiles) and S pairs
        cps = ctx.enter_context(tc.tile_pool(name="cps", bufs=2, space="PSUM"))
        sps = ctx.enter_context(tc.tile_pool(name="sps", bufs=2, space="PSUM"))

        # ---------------- input DMAs (need-ordered) ----------------
        def ring(i):
            return nc.sync if i % 2 == 0 else nc.scalar

        def load_wqk(jt):
            ring(jt).dma_start(out=wqk_sb[:, jt], in_=wqk_d[:, jt])

        # first head-pair's qk weights, then x^T for batch 0
        load_wqk(0)
        load_wqk(CT)
        for ct in range(CT):
            ring(ct).dma_start(
                out=xT_sb[:, ct, 0:N], in_=xT_d[ct * 128:(ct + 1) * 128, 0:N]
            )
        for ct in range(CT):
            ring(ct).dma_start(
                out=wv_sb[:, ct, :], in_=wv_d[ct * 128:(ct + 1) * 128, :]
            )
        for ct in range(CT):
            ring(ct).dma_start(
                out=xT_sb[:, ct, N:T], in_=xT_d[ct * 128:(ct + 1) * 128, N:T]
            )
        load_wqk(1)
        load_wqk(CT + 1)
        nc.sync.dma_start(out=qb_sb[:], in_=qb_d)
        nc.scalar.dma_start(out=vb_sb[:], in_=vb_d)
        # ones column of v_aug (denominator trick): partition-0 output row
        nc.gpsimd.memset(v_sb[:, :, :, 0:1], 1.0)
        # zero qk pad column: tail S matmuls run 66 queries wide so the
        # downstream bias-mult APs stay 4B-aligned (DVE 2x mode)
        nc.gpsimd.memset(qk_sb[:, :, :, N:NP], 0.0)
        # ot pad column: keeps pad-token proj rows finite
        nc.gpsimd.memset(ot_sb[:, :, :, N:NP], 0.0)

        def load_bt(hp):
            """exp(bias)^T for head pair hp: [key-part, par, key-tile, q]."""
            bt = btp.tile([128, 2, NT_M, NBP], bf16)
            for par in range(2):
                h = 2 * hp + par
                ring(par).dma_start(
                    out=bt[:, par, 0:4, 0:NP],
                    in_=bt_d[h, 0:512, 0:NP].rearrange("(a p) n -> p a n", p=128),
                )
                ring(par + 1).dma_start(
                    out=bt[0:65, par, 4, 0:NP], in_=bt_d[h, 512:N, 0:NP]
                )
            return bt

        # ---------------- chains ----------------
        def qk_chain(jt, b):
            ps = cps.tile([128, 1024], f32, tag="c")
            for ct in range(CT):
                for (q0, qn) in _chunks(N):
                    nc.tensor.matmul(
                        ps[:, q0:q0 + qn],
                        lhsT=wqk_sb[:, jt, ct, :],
                        rhs=xT_sb[:, ct, b * N + q0:b * N + q0 + qn],
                        start=(ct == 0),
                        stop=(ct == CT - 1),
                    )
            # chain drains split DVE/Scalar (one each per slot on average)
            if jt < CT:
                nc.vector.tensor_scalar(
                    out=qk_sb[:, jt, b, 0:N],
                    in0=ps[:, 0:N],
                    scalar1=qb_sb[:, jt:jt + 1],
                    scalar2=None,
                    op0=ALU.add,
                )
            else:
                nc.scalar.activation(qk_sb[:, jt, b, 0:N], ps[:, 0:N], ACTF.Copy)

        vb_v = vb_sb[:].rearrange("p (h d) -> p h d", d=HD)

        def v_chain(b, mt):
            mp = min(128, N - mt * 128)
            t0 = b * N + mt * 128
            ps = cps.tile([128, 1024], f32, tag="c")
            for ct in range(CT):
                for (j0, jn) in _chunks(C):
                    nc.tensor.matmul(
                        ps[0:mp, j0:j0 + jn],
                        lhsT=xT_sb[:, ct, t0:t0 + mp],
                        rhs=wv_sb[:, ct, j0:j0 + jn],
                        start=(ct == 0),
                        stop=(ct == CT - 1),
                    )
            nc.vector.tensor_add(
                v_sb[0:mp, b * NT_M + mt, :, 1:HD + 1],
                ps[0:mp, 0:C].rearrange("p (h d) -> p h d", d=HD),
                vb_v[0:mp],
            )

        def load_wp():
            nc.scalar.dma_start(
                out=wp_sb[:], in_=wp_d.rearrange("(a p) j -> p a j", p=128)
            )

        # ---------------- attention slot ----------------
        def slot(hp, b, bt, fillers):
            kT = qk_sb[:, CT + hp, b, :]
            qT = qk_sb[:, hp, b, :]
            # S^T per (key-tile, head): single-head [128, 578] PSUM tiles --
            # the 578-wide row splits exactly at the bank boundary
            # (512 | 66), so all 577 queries run in ONE pass (no tail pass,
            # which previously added a ~4us cross-engine latency appendix
            # to every slot).
            pts = []
            for mt in range(NT_M):
                mp = min(128, N - mt * 128)
                for par in range(2):
                    p0 = 64 * par
                    sp = sps.tile([128, 1024], f32, tag="sp")
                    for (q0, qn) in ((0, NQ0), (NQ0, NP - NQ0)):
                        nc.tensor.matmul(
                            sp[0:mp, q0:q0 + qn],
                            lhsT=kT[p0:p0 + 64, mt * 128:mt * 128 + mp],
                            rhs=qT[p0:p0 + 64, q0:q0 + qn],
                            start=True,
                            stop=True,
                            tile_position=(p0, 0),
                        )
                    pt = ptp.tile([128, NBP], bf16)
                    nc.scalar.activation(pt[0:mp, 0:NP], sp[0:mp, 0:NP], ACTF.Exp)
                    # bias mult: split DVE/Pool to balance engine load
                    eng = nc.vector if (2 * mt + par) % 5 < 3 else nc.gpsimd
                    eng.tensor_mul(
                        pt[0:mp, 0:NP], pt[0:mp, 0:NP], bt[0:mp, par, mt, 0:NP]
                    )
                    pts.append(pt)

            for f in fillers:
                f()

            # O'^T per head; PSUM partition 0 = softmax denominator
            scs = []
            rall = rap.tile([1, 2, NBP], f32)
            for par in range(2):
                ou = cps.tile([128, 1024], f32, name="ou", tag="c")
                for mt in range(NT_M):
                    mp = min(128, N - mt * 128)
                    for (q0, qn) in ((0, NQ0), (NQ0, N - NQ0)):
                        nc.tensor.matmul(
                            ou[0:HD + 1, q0:q0 + qn],
                            lhsT=v_sb[0:mp, b * NT_M + mt, 2 * hp + par, 0:HD + 1],
                            rhs=pts[2 * mt + par][0:mp, q0:q0 + qn],
                            start=(mt == 0),
                            stop=(mt == NT_M - 1),
                        )
                # reciprocal of the denominator row (PSUM partition 0)
                nc.vector.reciprocal_approx_fast(
                    rall[0:1, par, 0:N], ou[0:1, 0:N]
                )
                # PSUM -> SBUF drain (DVE casts f32->bf16)
                sc = scp.tile([128, NBP], bf16)
                nc.vector.tensor_copy(sc[0:HD + 1, 0:N], ou[0:HD + 1, 0:N])
                scs.append(sc)
            rd = rdp.tile([2, NBP], f32)
            nc.sync.dma_start(out=rd[:, 0:N], in_=rall[0:1, :, 0:N])
            bc = bcp.tile([128, NBP], f32)
            for par in range(2):
                p0 = 64 * par
                ring(par).dma_start(
                    out=bc[p0:p0 + 64, 0:N],
                    in_=rd[par:par + 1, 0:N].broadcast_to([64, N]),
                )
                # head-half reorder: sc rows 1:65 -> ot partitions
                ring(par).dma_start(
                    out=ot_sb[p0:p0 + 64, hp, b, 0:N], in_=scs[par][1:HD + 1, 0:N]
                )
            # normalize in place on Pool (SBUF-only engine)
            nc.gpsimd.tensor_mul(
                ot_sb[:, hp, b, 0:N], ot_sb[:, hp, b, 0:N], bc[:, 0:N]
            )

        # ---------------- emission schedule ----------------
        # prologue chains: everything slot (0,0) and (0,1) need
        qk_chain(0, 0)
        qk_chain(CT, 0)
        for mt in range(NT_M):
            v_chain(0, mt)
        bts = [load_bt(0), load_bt(1)]
        for jt in range(2, CT):
            load_wqk(jt)
            load_wqk(CT + jt)
        nc.sync.dma_start(out=pb_sb[:], in_=pb_d)

        # chains for slot k are emitted as filler inside slot k-2
        fillers = {
            0: [lambda: qk_chain(0, 1), lambda: qk_chain(CT, 1),
                lambda: v_chain(1, 0), lambda: v_chain(1, 1),
                lambda: v_chain(1, 2), lambda: qk_chain(1, 0),
                lambda: qk_chain(CT + 1, 0)],
            1: [lambda: v_chain(1, 3), lambda: v_chain(1, 4),
                lambda: qk_chain(1, 1), lambda: qk_chain(CT + 1, 1)],
        }
        for s in range(2, 10):
            hp_n, b_n = (s + 2) // 2, (s + 2) % 2
            jt = hp_n
            fillers[s] = [
                (lambda j=jt, bb=b_n: qk_chain(j, bb)),
                (lambda j=CT + jt, bb=b_n: qk_chain(j, bb)),
            ]
        fillers[6].append(load_wp)
        fillers[10] = []
        fillers[11] = []

        for s in range(12):
            hp, b = s // 2, s % 2
            if b == 0 and hp + 1 < HP and len(bts) <= hp + 1:
                bts.append(load_bt(hp + 1))
            slot(hp, b, bts[hp], fillers[s])

        # ---------------- output projection ----------------
        ot_flat = ot_sb[:].rearrange("p c b n -> p c (b n)")
        ntt = (TP + 127) // 128
        for tt in range(ntt):
            tp = min(128, TP - tt * 128)
            ps = cps.tile([128, 1024], f32, tag="c")
            for ct in range(CT):
                for (j0, jn) in _chunks(C):
                    nc.tensor.matmul(
                        ps[0:tp, j0:j0 + jn],
                        lhsT=ot_flat[:, ct, tt * 128:tt * 128 + tp],
                        rhs=wp_sb[:, ct, j0:j0 + jn],
                        start=(ct == 0),
                        stop=(ct == CT - 1),
                    )
            os = outp.tile([128, C], bf16)
            nc.vector.tensor_add(os[0:tp], ps[0:tp, 0:C], pb_sb[0:tp])
            ring(tt).dma_start(
                out=out_d[tt * 128:tt * 128 + tp, :], in_=os[0:tp]
            )

    nc.compile()
    return nc


def _get_nc():
    if "nc" not in _CACHE:
        _CACHE["nc"] = _build()
    return _CACHE["nc"]


def _prep_inputs(x, relative_position_bias, qkv_weight, q_bias, v_bias,
                 proj_weight, proj_bias):
    """Host-side layout prep + per-core sharding. Returns list of in_maps."""
    import ml_dtypes

    f = np.float32
    bf = ml_dtypes.bfloat16
    x = np.asarray(x, f)
    bias = np.asarray(relative_position_bias, f)
    w = np.asarray(qkv_weight, f)
    qb = np.asarray(q_bias, f)
    vb = np.asarray(v_bias, f)
    wp = np.asarray(proj_weight, f)
    pb = np.asarray(proj_bias, f)

    wq_s = w[0:C] * f(SCALE)            # fold q scaling into weights/bias
    qb_s = qb * f(SCALE)
    wqkT = np.concatenate([wq_s, w[C:2 * C]], 0).T.astype(bf)   # [C, 2C]
    # partition-major jt-blocked layout: [p, jt, ct, jc]
    wqkH = np.ascontiguousarray(
        wqkT.reshape(CT, 128, 2 * CT, 128).transpose(1, 2, 0, 3)
    )
    wvT = np.ascontiguousarray(w[2 * C:].T.astype(bf))
    wpT = np.ascontiguousarray(wp.T.astype(bf))
    qbT = np.ascontiguousarray(qb_s.reshape(CT, 128).T)
    vbB = np.ascontiguousarray(np.broadcast_to(vb, (128, C)))
    pbB = np.ascontiguousarray(np.broadcast_to(pb, (128, C)))
    BT = np.zeros((H, N, NBP), dtype=bf)  # cols 577..579 stay 0
    BT[:, :, 0:N] = np.exp(bias.transpose(0, 2, 1), dtype=np.float32).astype(bf)

    shared = dict(wqkH=wqkH, wvT=wvT, wpT=wpT, qbT=qbT, vbB=vbB, pbB=pbB, BT=BT)
    in_maps = []
    for c in range(NCORES):
        xs = x[c * BL:(c + 1) * BL].reshape(T, C)
        in_maps.append(dict(shared, xT=np.ascontiguousarray(xs.T.astype(bf))))
    return in_maps


def kernel(x, relative_position_bias, qkv_weight, q_bias, v_bias,
           proj_weight, proj_bias):
    from concourse import bass_utils

    in_maps = _prep_inputs(x, relative_position_bias, qkv_weight, q_bias,
                           v_bias, proj_weight, proj_bias)
    nc = _get_nc()
    res = bass_utils.run_bass_kernel_spmd(nc, in_maps, core_ids=list(range(NCORES)))
    out = np.concatenate(
        [res.results[c]["out"].reshape(BL, NP, C)[:, :N, :] for c in range(NCORES)],
        axis=0,
    )
    return out.astype(np.float32)


# revision 20
# speedup vs baseline: 1.0660x; 1.0660x over previous
"""Trainium2 Bass kernel for nn_Attention (ViT-style attention block).

Reference computation (per batch b, head h):
    qkv  = x @ qkv_weight.T + [q_bias, 0, v_bias]        # [B,N,3C]
    q,k,v split into heads of HD=64;  q *= HD**-0.5
    S    = q @ k.T + relative_position_bias[h]           # [N,N]
    P    = softmax(S, axis=-1)
    O    = P @ v                                         # [N,HD]
    out  = concat_heads(O) @ proj_weight.T + proj_bias   # [B,N,C]

Sharding: pure data-parallel over batch: 16 batches -> 2 per core across
8 NeuronCores; weights replicated; outputs concatenated on the host.

Single software-pipelined instruction stream (v2): qkv projection chains,
attention head-pair slots and the output projection are emitted in one
interleaved order so the PE never idles long enough to drop out of its
full-speed p-state (PE runs at 1.2GHz for ~3us after any idle gap; the
baseline's phase-separated version paid ~2x on every attention matmul).

Per attention slot (head-pair hp, batch b), queries split 512 + 65 so
every PSUM access stays inside one 2KB bank:
  - S^T pair matmuls (K=64) for both heads at tile_position (0,0)/(64,0).
  - ONE exp on ScalarE per mt covers both heads; P^T *= exp(bias^T)
    (host-precomputed) split between DVE and Pool.
  - O'^T accumulates over key tiles; v_aug = [1 | v] puts the softmax
    denominator in PSUM partition 0, so DVE reciprocal reads it directly
    (no SWDGE collect); f32 HWDGE bounce + partition-broadcast; normalization is an in-place multiply on Pool.
  - qkv/proj chains for FUTURE slots are emitted inside each slot as PE
    filler while ScalarE (exp, the per-slot rate limiter) catches up.

PSUM budget (8 banks): chain pool 2x[128,1024]f32 (4) + S-pair pool
2x[128,2,512]f32 (4); O accumulators borrow chain-pool tiles.

bf16 DMA rule learned on HW: a DMA write run into SBUF must start 4-byte
aligned and may overhang its end by 2 bytes -- all bf16 destination rows
here are padded so overhangs land in padding (580-wide bias rows, 578-wide
ot rows, 66-wide tail tiles).
"""

import numpy as np

B, N, C = 16, 577, 768
H, HD = 12, 64
SCALE = HD ** -0.5
NCORES = 8
BL = B // NCORES           # batches per core (2)
T = BL * N                 # tokens per core (1154)
NT_M = (N + 127) // 128    # key tiles per batch (5: 4x128 + 65)
CT = C // 128              # 128-contraction tiles over C (6)
HP = H // 2                # head pairs (6)
NP = N + 1                 # padded query stride (578)
TP = BL * NP               # padded token rows (1156)
NQ0 = 512                  # main-pass queries per batch
NQ1 = N - NQ0              # tail-pass queries (65)
NQ1W = 66                  # tail width incl pad query (4B-aligned blocks)
NBP = 580                  # bias row stride (577 padded to 4B-aligned even)

_CACHE = {}


def _chunks(total, limit=512):
    out = []
    pos = 0
    while pos < total:
        n = min(limit, total - pos)
        out.append((pos, n))
        pos += n
    return out


def _build():
    """Trace the Bass/Tile program once. Returns the Bass object."""
    import concourse.bass as bass
    import concourse.tile as tile
    from concourse import bacc, mybir
    from contextlib import ExitStack

    f32 = mybir.dt.float32
    bf16 = mybir.dt.bfloat16
    ALU = mybir.AluOpType
    ACTF = mybir.ActivationFunctionType

    nc = bacc.Bacc("TRN2", target_bir_lowering=False, debug=False)

    # ---- DRAM I/O ----
    xT_d = nc.dram_tensor("xT", [C, T], bf16, kind="ExternalInput").ap()
    # qk weights partition-major + jt-block-major: one contiguous DMA per jt
    wqk_d = nc.dram_tensor(
        "wqkH", [128, 2 * CT, CT, 128], bf16, kind="ExternalInput"
    ).ap()
    wv_d = nc.dram_tensor("wvT", [C, C], bf16, kind="ExternalInput").ap()
    wp_d = nc.dram_tensor("wpT", [C, C], bf16, kind="ExternalInput").ap()
    qb_d = nc.dram_tensor("qbT", [128, CT], f32, kind="ExternalInput").ap()
    vb_d = nc.dram_tensor("vbB", [128, C], f32, kind="ExternalInput").ap()
    pb_d = nc.dram_tensor("pbB", [128, C], f32, kind="ExternalInput").ap()
    bt_d = nc.dram_tensor("BT", [H, N, NBP], bf16, kind="ExternalInput").ap()
    out_d = nc.dram_tensor("out", [TP, C], bf16, kind="ExternalOutput").ap()

    with tile.TileContext(nc) as tc, ExitStack() as ctx:
        const = ctx.enter_context(tc.tile_pool(name="const", bufs=1))
        persist = ctx.enter_context(tc.tile_pool(name="persist", bufs=1))

        qb_sb = const.tile([128, CT], f32)
        vb_sb = const.tile([128, C], f32)
        pb_sb = const.tile([128, C], f32)

        xT_sb = persist.tile([128, CT, T], bf16)
        wqk_sb = persist.tile([128, 2 * CT, CT, 128], bf16)
        wv_sb = persist.tile([128, CT, C], bf16)
        wp_sb = persist.tile([128, CT, C], bf16)
        qk_sb = persist.tile([128, 2 * CT, BL, NP], bf16)   # q^T | k^T
        v_sb = persist.tile([128, BL * NT_M, H, HD + 2], bf16)  # [1|v|pad]
        ot_sb = persist.tile([128, HP, BL, NP], bf16)       # O^T normalized

        # SBUF pools
        btp = ctx.enter_context(tc.tile_pool(name="btp", bufs=2))
        ptp = ctx.enter_context(tc.tile_pool(name="ptp", bufs=8))
        pttp = ctx.enter_context(tc.tile_pool(name="pttp", bufs=2))
        scp = ctx.enter_context(tc.tile_pool(name="scp", bufs=2))
        sctp = ctx.enter_context(tc.tile_pool(name="sctp", bufs=2))
        rap = ctx.enter_context(tc.tile_pool(name="rap", bufs=2))
        rdp = ctx.enter_context(tc.tile_pool(name="rdp", bufs=2, space="DRAM"))
        bcp = ctx.enter_context(tc.tile_pool(name="bcp", bufs=3))
        outp = ctx.enter_context(tc.tile_pool(name="outp", bufs=3))

        # PSUM pools: chains+O accumulators (2-bank tiles) and S pairs
        cps = ctx.enter_context(tc.tile_pool(name="cps", bufs=2, space="PSUM"))
        sps = ctx.enter_context(tc.tile_pool(name="sps", bufs=2, space="PSUM"))

        # ---------------- input DMAs (need-ordered) ----------------
        def ring(i):
            return nc.sync if i % 2 == 0 else nc.scalar

        def load_wqk(jt):
            ring(jt).dma_start(out=wqk_sb[:, jt], in_=wqk_d[:, jt])

        # first head-pair's qk weights, then x^T for batch 0
        load_wqk(0)
        load_wqk(CT)
        for ct in range(CT):
            ring(ct).dma_start(
                out=xT_sb[:, ct, 0:N], in_=xT_d[ct * 128:(ct + 1) * 128, 0:N]
            )
        for ct in range(CT):
            ring(ct).dma_start(
                out=wv_sb[:, ct, :], in_=wv_d[ct * 128:(ct + 1) * 128, :]
            )
        for ct in range(CT):
            ring(ct).dma_start(
                out=xT_sb[:, ct, N:T], in_=xT_d[ct * 128:(ct + 1) * 128, N:T]
            )
        load_wqk(1)
        load_wqk(CT + 1)
        nc.sync.dma_start(out=qb_sb[:], in_=qb_d)
        nc.scalar.dma_start(out=vb_sb[:], in_=vb_d)
        # ones column of v_aug (denominator trick): partition-0 output row
        nc.gpsimd.memset(v_sb[:, :, :, 0:1], 1.0)
        # zero qk pad column: tail S matmuls run 66 queries wide so the
        # downstream bias-mult APs stay 4B-aligned (DVE 2x mode)
        nc.gpsimd.memset(qk_sb[:, :, :, N:NP], 0.0)
        # ot pad column: keeps pad-token proj rows finite
        nc.gpsimd.memset(ot_sb[:, :, :, N:NP], 0.0)

        def load_bt(hp):
            """exp(bias)^T for head pair hp: [key-part, par, key-tile, q]."""
            bt = btp.tile([128, 2, NT_M, NBP], bf16)
            for par in range(2):
                h = 2 * hp + par
                ring(par).dma_start(
                    out=bt[:, par, 0:4, 0:NP],
                    in_=bt_d[h, 0:512, 0:NP].rearrange("(a p) n -> p a n", p=128),
                )
                ring(par + 1).dma_start(
                    out=bt[0:65, par, 4, 0:NP], in_=bt_d[h, 512:N, 0:NP]
                )
            return bt

        # ---------------- chains ----------------
        def qk_chain(jt, b):
            ps = cps.tile([128, 1024], f32, tag="c")
            for ct in range(CT):
                for (q0, qn) in _chunks(N):
                    nc.tensor.matmul(
                        ps[:, q0:q0 + qn],
                        lhsT=wqk_sb[:, jt, ct, :],
                        rhs=xT_sb[:, ct, b * N + q0:b * N + q0 + qn],
                        start=(ct == 0),
                        stop=(ct == CT - 1),
                    )
            # chain drains split DVE/Scalar (one each per slot on average)
            if jt < CT:
                nc.vector.tensor_scalar(
                    out=qk_sb[:, jt, b, 0:N],
                    in0=ps[:, 0:N],
                    scalar1=qb_sb[:, jt:jt + 1],
                    scalar2=None,
                    op0=ALU.add,
                )
            else:
                nc.scalar.activation(qk_sb[:, jt, b, 0:N], ps[:, 0:N], ACTF.Copy)

        vb_v = vb_sb[:].rearrange("p (h d) -> p h d", d=HD)

        def v_chain(b, mt):
            mp = min(128, N - mt * 128)
            t0 = b * N + mt * 128
            ps = cps.tile([128, 1024], f32, tag="c")
            for ct in range(CT):
                for (j0, jn) in _chunks(C):
                    nc.tensor.matmul(
                        ps[0:mp, j0:j0 + jn],
                        lhsT=xT_sb[:, ct, t0:t0 + mp],
                        rhs=wv_sb[:, ct, j0:j0 + jn],
                        start=(ct == 0),
                        stop=(ct == CT - 1),
                    )
            nc.vector.tensor_add(
                v_sb[0:mp, b * NT_M + mt, :, 1:HD + 1],
                ps[0:mp, 0:C].rearrange("p (h d) -> p h d", d=HD),
                vb_v[0:mp],
            )

        def load_wp():
            nc.scalar.dma_start(
                out=wp_sb[:], in_=wp_d.rearrange("(a p) j -> p a j", p=128)
            )

        # ---------------- attention slot ----------------
        # Per (head-pair, batch): S^T pair matmuls into [128,2,512] PSUM
        # tiles (queries 0:512), ONE exp per key-tile covers both heads.
        # The 65-query tail pass is DEFERRED into the NEXT slot: its
        # S-tail -> exp -> mult -> O-tail chain snakes across three busy
        # engines (~4us latency) and would otherwise sit on the PE's
        # in-order queue between slots.
        state = {}

        def main_part(hp, b, bt, fillers):
            kT = qk_sb[:, CT + hp, b, :]
            qT = qk_sb[:, hp, b, :]
            pts = []
            for mt in range(NT_M):
                mp = min(128, N - mt * 128)
                sp = sps.tile([128, 2, NQ0], f32, tag="sp")
                for par in range(2):
                    p0 = 64 * par
                    nc.tensor.matmul(
                        sp[0:mp, par, :],
                        lhsT=kT[p0:p0 + 64, mt * 128:mt * 128 + mp],
                        rhs=qT[p0:p0 + 64, 0:NQ0],
                        start=True,
                        stop=True,
                        tile_position=(p0, 0),
                    )
                pt = ptp.tile([128, 2, NQ0], bf16)
                nc.scalar.activation(pt[0:mp], sp[0:mp], ACTF.Exp)
                # bias mult: split DVE/Pool to balance engine load
                eng = nc.vector if mt % 2 == 0 else nc.gpsimd
                eng.tensor_mul(pt[0:mp], pt[0:mp], bt[0:mp, :, mt, 0:NQ0])
                pts.append(pt)

            for f in fillers:
                f()

            # O main accumulation; partition 0 = softmax denominator
            ou = cps.tile([128, 1024], f32, name="ou", tag="c")
            ou = ou.rearrange("p (a n) -> p a n", n=NQ0)
            for mt in range(NT_M):
                mp = min(128, N - mt * 128)
                for par in range(2):
                    nc.tensor.matmul(
                        ou[0:HD + 1, par, :],
                        lhsT=v_sb[0:mp, b * NT_M + mt, 2 * hp + par, 0:HD + 1],
                        rhs=pts[mt][0:mp, par, :],
                        start=(mt == 0),
                        stop=(mt == NT_M - 1),
                    )
            rall = rap.tile([1, 2, NBP], f32)
            nc.vector.reciprocal_approx_fast(rall[0:1, :, 0:NQ0], ou[0:1, :, :])
            sc = scp.tile([128, 2, NQ0], bf16)
            nc.vector.tensor_copy(sc[0:HD + 1], ou[0:HD + 1])
            for par in range(2):
                p0 = 64 * par
                ring(par).dma_start(
                    out=ot_sb[p0:p0 + 64, hp, b, 0:NQ0], in_=sc[1:HD + 1, par, :]
                )
            state[(hp, b)] = (bt, rall)

        def tail_part(hp, b):
            """Queries 512:578 for slot (hp,b), run inside the NEXT slot."""
            bt, rall = state.pop((hp, b))
            kT = qk_sb[:, CT + hp, b, :]
            qT = qk_sb[:, hp, b, :]
            spt = sps.tile([128, 2, NQ0], f32, tag="sp")
            for mt in range(NT_M):
                mp = min(128, N - mt * 128)
                for par in range(2):
                    p0 = 64 * par
                    nc.tensor.matmul(
                        spt[0:mp, par, mt * NQ1W:mt * NQ1W + NQ1W],
                        lhsT=kT[p0:p0 + 64, mt * 128:mt * 128 + mp],
                        rhs=qT[p0:p0 + 64, NQ0:NQ0 + NQ1W],
                        start=True,
                        stop=True,
                        tile_position=(p0, 0),
                    )
            ptt = pttp.tile([128, 2, NT_M * NQ1W], bf16)
            ptt_v = ptt[:].rearrange("p a (m q) -> p a m q", q=NQ1W)
            nc.scalar.activation(
                ptt[:, :, 0:4 * NQ1W], spt[:, :, 0:4 * NQ1W], ACTF.Exp
            )
            nc.scalar.activation(
                ptt[0:65, :, 4 * NQ1W:5 * NQ1W],
                spt[0:65, :, 4 * NQ1W:5 * NQ1W],
                ACTF.Exp,
            )
            nc.vector.tensor_mul(
                ptt_v[:, :, 0:4, :], ptt_v[:, :, 0:4, :],
                bt[:, :, 0:4, NQ0:NQ0 + NQ1W],
            )
            nc.vector.tensor_mul(
                ptt_v[0:65, :, 4, :], ptt_v[0:65, :, 4, :],
                bt[0:65, :, 4, NQ0:NQ0 + NQ1W],
            )
            out_t = cps.tile([128, 1024], f32, name="out_t", tag="c")
            out_t = out_t.rearrange("p (a n) -> p a n", n=NQ0)
            for mt in range(NT_M):
                mp = min(128, N - mt * 128)
                for par in range(2):
                    nc.tensor.matmul(
                        out_t[0:HD + 1, par, 0:NQ1],
                        lhsT=v_sb[0:mp, b * NT_M + mt, 2 * hp + par, 0:HD + 1],
                        rhs=ptt[0:mp, par, mt * NQ1W:mt * NQ1W + NQ1],
                        start=(mt == 0),
                        stop=(mt == NT_M - 1),
                    )
            nc.vector.reciprocal_approx_fast(
                rall[0:1, :, NQ0:N], out_t[0:1, :, 0:NQ1]
            )
            sct = sctp.tile([128, 2, HD + 2], bf16)
            nc.vector.tensor_copy(
                sct[0:HD + 1, :, 0:NQ1], out_t[0:HD + 1, :, 0:NQ1]
            )
            rd = rdp.tile([2, NBP], f32)
            nc.sync.dma_start(out=rd[:, 0:N], in_=rall[0:1, :, 0:N])
            bc = bcp.tile([128, NBP], f32)
            for par in range(2):
                p0 = 64 * par
                ring(par).dma_start(
                    out=bc[p0:p0 + 64, 0:N],
                    in_=rd[par:par + 1, 0:N].broadcast_to([64, N]),
                )
                ring(par + 1).dma_start(
                    out=ot_sb[p0:p0 + 64, hp, b, NQ0:N],
                    in_=sct[1:HD + 1, par, 0:NQ1],
                )
            # normalize in place on Pool (SBUF-only engine)
            nc.gpsimd.tensor_mul(
                ot_sb[:, hp, b, 0:N], ot_sb[:, hp, b, 0:N], bc[:, 0:N]
            )

        # ---------------- emission schedule ----------------
        # prologue chains: everything slot (0,0) and (0,1) need
        qk_chain(0, 0)
        qk_chain(CT, 0)
        for mt in range(NT_M):
            v_chain(0, mt)
        bts = [load_bt(0), load_bt(1)]
        for jt in range(2, CT):
            load_wqk(jt)
            load_wqk(CT + jt)
        nc.sync.dma_start(out=pb_sb[:], in_=pb_d)

        # chains for slot k are emitted as filler inside slot k-2
        fillers = {
            0: [lambda: qk_chain(0, 1), lambda: qk_chain(CT, 1),
                lambda: v_chain(1, 0), lambda: v_chain(1, 1),
                lambda: v_chain(1, 2), lambda: qk_chain(1, 0),
                lambda: qk_chain(CT + 1, 0)],
            1: [lambda: v_chain(1, 3), lambda: v_chain(1, 4),
                lambda: qk_chain(1, 1), lambda: qk_chain(CT + 1, 1)],
        }
        for s in range(2, 10):
            hp_n, b_n = (s + 2) // 2, (s + 2) % 2
            jt = hp_n
            fillers[s] = [
                (lambda j=jt, bb=b_n: qk_chain(j, bb)),
                (lambda j=CT + jt, bb=b_n: qk_chain(j, bb)),
            ]
        fillers[6].append(load_wp)
        fillers[10] = []
        fillers[11] = []

        for s in range(12):
            hp, b = s // 2, s % 2
            if b == 0 and hp + 1 < HP and len(bts) <= hp + 1:
                bts.append(load_bt(hp + 1))
            main_part(hp, b, bts[hp], fillers[s])
            if s > 0:
                tail_part((s - 1) // 2, (s - 1) % 2)
        tail_part(5, 1)

        # ---------------- output projection ----------------
        ot_flat = ot_sb[:].rearrange("p c b n -> p c (b n)")
        ntt = (TP + 127) // 128
        for tt in range(ntt):
            tp = min(128, TP - tt * 128)
            ps = cps.tile([128, 1024], f32, tag="c")
            for ct in range(CT):
                for (j0, jn) in _chunks(C):
                    nc.tensor.matmul(
                        ps[0:tp, j0:j0 + jn],
                        lhsT=ot_flat[:, ct, tt * 128:tt * 128 + tp],
                        rhs=wp_sb[:, ct, j0:j0 + jn],
                        start=(ct == 0),
                        stop=(ct == CT - 1),
                    )
            os = outp.tile([128, C], bf16)
            nc.vector.tensor_add(os[0:tp], ps[0:tp, 0:C], pb_sb[0:tp])
            ring(tt).dma_start(
                out=out_d[tt * 128:tt * 128 + tp, :], in_=os[0:tp]
            )

    nc.compile()
    return nc


def _get_nc():
    if "nc" not in _CACHE:
        _CACHE["nc"] = _build()
    return _CACHE["nc"]


def _prep_inputs(x, relative_position_bias, qkv_weight, q_bias, v_bias,
                 proj_weight, proj_bias):
    """Host-side layout prep + per-core sharding. Returns list of in_maps."""
    import ml_dtypes

    f = np.float32
    bf = ml_dtypes.bfloat16
    x = np.asarray(x, f)
    bias = np.asarray(relative_position_bias, f)
    w = np.asarray(qkv_weight, f)
    qb = np.asarray(q_bias, f)
    vb = np.asarray(v_bias, f)
    wp = np.asarray(proj_weight, f)
    pb = np.asarray(proj_bias, f)

    wq_s = w[0:C] * f(SCALE)            # fold q scaling into weights/bias
    qb_s = qb * f(SCALE)
    wqkT = np.concatenate([wq_s, w[C:2 * C]], 0).T.astype(bf)   # [C, 2C]
    # partition-major jt-blocked layout: [p, jt, ct, jc]
    wqkH = np.ascontiguousarray(
        wqkT.reshape(CT, 128, 2 * CT, 128).transpose(1, 2, 0, 3)
    )
    wvT = np.ascontiguousarray(w[2 * C:].T.astype(bf))
    wpT = np.ascontiguousarray(wp.T.astype(bf))
    qbT = np.ascontiguousarray(qb_s.reshape(CT, 128).T)
    vbB = np.ascontiguousarray(np.broadcast_to(vb, (128, C)))
    pbB = np.ascontiguousarray(np.broadcast_to(pb, (128, C)))
    BT = np.zeros((H, N, NBP), dtype=bf)  # cols 577..579 stay 0
    BT[:, :, 0:N] = np.exp(bias.transpose(0, 2, 1), dtype=np.float32).astype(bf)

    shared = dict(wqkH=wqkH, wvT=wvT, wpT=wpT, qbT=qbT, vbB=vbB, pbB=pbB, BT=BT)
    in_maps = []
    for c in range(NCORES):
        xs = x[c * BL:(c + 1) * BL].reshape(T, C)
        in_maps.append(dict(shared, xT=np.ascontiguousarray(xs.T.astype(bf))))
    return in_maps


def kernel(x, relative_position_bias, qkv_weight, q_bias, v_bias,
           proj_weight, proj_bias):
    from concourse import bass_utils

    in_maps = _prep_inputs(x, relative_position_bias, qkv_weight, q_bias,
                           v_bias, proj_weight, proj_bias)
    nc = _get_nc()
    res = bass_utils.run_bass_kernel_spmd(nc, in_maps, core_ids=list(range(NCORES)))
    out = np.concatenate(
        [res.results[c]["out"].reshape(BL, NP, C)[:, :N, :] for c in range(NCORES)],
        axis=0,
    )
    return out.astype(np.float32)
